# revision 2
# baseline (speedup 1.0000x reference)
"""Trainium2 Bass kernel: single-head causal attention (B=4, S=2048, D=1024).

reference:
  K = Xk @ WK; Q = Xq @ WQ; V = Xv @ WV          [B,S,D] @ [D,D]
  out = softmax(causal(Q K^T / sqrt(D))) @ V      [B,S,D]

Sharding over 8 NeuronCores (one SPMD program):
  core c -> (batch b = c//2, e-half h = c%2)
  Each core computes its batch's full K and Q projections and the full causal
  softmax, but only its 512-wide slice of V / output (WV pre-sliced on host).
  This keeps the static schedule identical on every core (SPMD) while
  exploiting causality fully, per the spec's "tensor-parallel over
  output_size" hint (the all-gather is just the host-side concat of the two
  output halves).

Per-core pipeline (fp16 matmuls on the PE, fp32 PSUM + fp32 softmax):
  Phase A: PE-transpose X tiles (fp32), project
           K^T, Q^T -> [e, s] layout (W stationary),
           V (e-half) -> [s, e] natural (X^T stationary).
  Phase B: per 128-query block qb (16 blocks, nk = qb+1 visible key tiles):
           scores = Q^T.T K^T (8 accumulating matmuls per 512-key chunk),
           causal mask on the diagonal tile, row-max (negated) on DVE,
           p = exp(scores/sqrt(D) - max/sqrt(D)) on ACT with fp32 row sums,
           PE-transpose p tiles, out = p^T.T @ V accumulated over key tiles,
           normalize by 1/rowsum, DMA out.
"""
import numpy as np

B, S, D = 4, 2048, 1024
P = 128
SB = S // P            # 16 key/query blocks
DC = D // P            # 8 contraction chunks of 128
EB = D // P            # 8 e-blocks of 128
EHALF = D // 2         # 512: per-core e-slice of V / output
INV_SQRT_D = float(1.0 / np.sqrt(np.float64(D)))
NCORES = 8

_CACHE = {}


def _build_nc():
    import concourse.bacc as bacc
    import concourse.mybir as mybir
    import concourse.tile as tile
    from concourse.masks import make_causal_mask, make_identity
    from contextlib import ExitStack

    fp32 = mybir.dt.float32
    fp16 = mybir.dt.float16
    Exp = mybir.ActivationFunctionType.Exp
    Add = mybir.AluOpType.add
    Max = mybir.AluOpType.max
    X = mybir.AxisListType.X

    nc = bacc.Bacc("TRN2", target_bir_lowering=False, debug=False,
                   num_devices=NCORES)

    xk_d = nc.dram_tensor("xk", [S, D], fp32, kind="ExternalInput")
    xv_d = nc.dram_tensor("xv", [S, D], fp32, kind="ExternalInput")
    xq_d = nc.dram_tensor("xq", [S, D], fp32, kind="ExternalInput")
    wk_d = nc.dram_tensor("wk", [D, D], fp32, kind="ExternalInput")
    wq_d = nc.dram_tensor("wq", [D, D], fp32, kind="ExternalInput")
    wv_d = nc.dram_tensor("wv", [D, EHALF], fp32, kind="ExternalInput")
    o_d = nc.dram_tensor("o", [S, EHALF], fp32, kind="ExternalOutput")

    # round-robin the PSUM->SBUF copies between DVE and ACT so neither
    # becomes the bottleneck under the PE's ~280us of matmul work
    copy_ctr = [0]

    with tile.TileContext(nc) as tc:
        with ExitStack() as top:
            persist = top.enter_context(tc.tile_pool(name="persist", bufs=1))
            kt_h = persist.tile([P, EB, S], fp16, name="kt_h")
            qt_h = persist.tile([P, EB, S], fp16, name="qt_h")
            v_h = persist.tile([P, SB, EHALF], fp16, name="v_h")
            ident32 = persist.tile([P, P], fp32, name="ident32")
            make_identity(nc, ident32[:])
            ident16 = persist.tile([P, P], fp16, name="ident16")
            make_identity(nc, ident16[:])
            cmask = persist.tile([P, P], fp32, name="cmask")
            make_causal_mask(nc, cmask[:], mask_val=-1e30)

            def alt_copy(dst, src):
                # alternate engines 2:1 DVE:ACT (DVE copies are ~2-3x faster)
                i = copy_ctr[0]
                copy_ctr[0] += 1
                if i % 3 == 2:
                    nc.scalar.copy(dst, src)
                else:
                    nc.vector.tensor_copy(dst, src)

            # ---------------- Phase A: projections ----------------
            with ExitStack() as pa:
                wpool = pa.enter_context(tc.tile_pool(name="wpool", bufs=2))
                wstage = pa.enter_context(tc.tile_pool(name="wstage", bufs=1))
                xpool = pa.enter_context(tc.tile_pool(name="xpool", bufs=2))
                xtpool = pa.enter_context(tc.tile_pool(name="xtpool", bufs=2))
                psA = pa.enter_context(
                    tc.tile_pool(name="psA", bufs=3, space="PSUM"))
                psT = pa.enter_context(
                    tc.tile_pool(name="psT", bufs=3, space="PSUM"))

                def load_w(w_d, ecols, nm):
                    wh = wpool.tile([P, DC, D], fp16, name=nm, tag="w_h")
                    for half in range(ecols // EHALF):
                        ws = wstage.tile([P, DC, EHALF], fp32, name="ws",
                                         tag="ws")
                        nc.sync.dma_start(
                            ws[:],
                            w_d.rearrange("(c p) e -> p c e", p=P)[
                                :, :, half * EHALF:(half + 1) * EHALF])
                        nc.vector.tensor_copy(
                            wh[:, :, half * EHALF:(half + 1) * EHALF], ws[:])
                    return wh[:, :, :ecols]

                def load_xt(x_d, ch):
                    """Load rows [ch*512, (ch+1)*512) of x and return the
                    transposed fp16 tile [P(d), DC, 512(s)]."""
                    xn = xpool.tile([P, 4, D], fp32, name="xn", tag="xn")
                    nc.sync.dma_start(
                        xn[:],
                        x_d[ch * 512:(ch + 1) * 512].rearrange(
                            "(a p) d -> p a d", p=P))
                    xt = xtpool.tile([P, DC, 512], fp16, name="xt", tag="xt")
                    for a in range(4):
                        for dc in range(DC):
                            pst = psT.tile([P, P], fp32, name="pst", tag="pst")
                            nc.tensor.transpose(
                                pst[:], xn[:, a, dc * P:(dc + 1) * P],
                                ident32[:])
                            alt_copy(xt[:, dc, a * P:(a + 1) * P], pst[:])
                    return xt

                wk_h = load_w(wk_d, D, "wk_h")
                wq_h = load_w(wq_d, D, "wq_h")

                # K and Q projections: out[e, s] with W stationary
                for w_h, dst in ((wk_h, kt_h), (wq_h, qt_h)):
                    x_d = xk_d if dst is kt_h else xq_d
                    for ch in range(S // 512):
                        xt = load_xt(x_d, ch)
                        for eb in range(EB):
                            ps = psA.tile([P, 512], fp32, name="psa", tag="psa")
                            for dc in range(DC):
                                nc.tensor.matmul(
                                    ps[:],
                                    w_h[:, dc, eb * P:(eb + 1) * P],
                                    xt[:, dc, :],
                                    start=(dc == 0), stop=(dc == DC - 1))
                            alt_copy(dst[:, eb, ch * 512:(ch + 1) * 512],
                                     ps[:])

                # V projection (e-half): out[s, e] with X^T stationary
                wv_h = load_w(wv_d, EHALF, "wv_h")
                for ch in range(S // 512):
                    xt = load_xt(xv_d, ch)
                    for a in range(4):
                        ps = psA.tile([P, 512], fp32, name="psa", tag="psa")
                        for dc in range(DC):
                            nc.tensor.matmul(
                                ps[:],
                                xt[:, dc, a * P:(a + 1) * P],
                                wv_h[:, dc, :],
                                start=(dc == 0), stop=(dc == DC - 1))
                        alt_copy(v_h[:, ch * 4 + a, :], ps[:])

            # ---------------- Phase B: causal attention ----------------
            with ExitStack() as pb:
                spool = pb.enter_context(tc.tile_pool(name="spool", bufs=2))
                ppool = pb.enter_context(tc.tile_pool(name="ppool", bufs=2))
                ptpool = pb.enter_context(tc.tile_pool(name="ptpool", bufs=2))
                smpool = pb.enter_context(tc.tile_pool(name="smpool", bufs=3))
                opool = pb.enter_context(tc.tile_pool(name="opool", bufs=2))
                psBs = pb.enter_context(
                    tc.tile_pool(name="psBs", bufs=2, space="PSUM"))
                psBt = pb.enter_context(
                    tc.tile_pool(name="psBt", bufs=2, space="PSUM"))
                psBo = pb.enter_context(
                    tc.tile_pool(name="psBo", bufs=2, space="PSUM"))

                for gb in range(SB):
                    nk = gb + 1
                    kw = nk * P  # visible key width
                    scores = spool.tile([P, S], fp32, name="scores",
                                        tag="scores")
                    for c0 in range(0, kw, 512):
                        w = min(512, kw - c0)
                        ps = psBs.tile([P, 512], fp32, name="ps_s", tag="ps_s")
                        for dc in range(DC):
                            nc.tensor.matmul(
                                ps[:, :w],
                                qt_h[:, dc, gb * P:(gb + 1) * P],
                                kt_h[:, dc, c0:c0 + w],
                                start=(dc == 0), stop=(dc == DC - 1))
                        nc.vector.tensor_copy(scores[:, c0:c0 + w], ps[:, :w])

                    # causal mask on the diagonal tile
                    nc.vector.tensor_tensor(
                        scores[:, gb * P:kw], scores[:, gb * P:kw], cmask[:],
                        Add)

                    negmax = smpool.tile([P, 1], fp32, name="negmax",
                                         tag="negmax")
                    nc.vector.tensor_reduce(negmax[:], scores[:, :kw], X, Max,
                                            negate=True)
                    negmax_s = smpool.tile([P, 1], fp32, name="negmax_s",
                                           tag="negmax_s")
                    nc.vector.tensor_scalar_mul(negmax_s[:], negmax[:],
                                                INV_SQRT_D)

                    p16 = ppool.tile([P, S], fp16, name="p16", tag="p16")
                    sums = smpool.tile([P, 1], fp32, name="sums", tag="sums")
                    nc.scalar.activation(p16[:, :kw], scores[:, :kw], Exp,
                                         bias=negmax_s[:], scale=INV_SQRT_D,
                                         accum_out=sums[:])

                    pt = ptpool.tile([P, SB, P], fp16, name="pt", tag="pt")
                    for kc in range(nk):
                        pst = psBt.tile([P, P], fp16, name="ps_t", tag="ps_t")
                        nc.tensor.transpose(
                            pst[:], p16[:, kc * P:(kc + 1) * P], ident16[:])
                        nc.vector.tensor_copy(pt[:, kc], pst[:])

                    pso = psBo.tile([P, 512], fp32, name="ps_o", tag="ps_o")
                    for kc in range(nk):
                        nc.tensor.matmul(pso[:], pt[:, kc], v_h[:, kc, :],
                                         start=(kc == 0), stop=(kc == nk - 1))

                    recip = smpool.tile([P, 1], fp32, name="recip",
                                        tag="recip")
                    nc.vector.reciprocal(recip[:], sums[:])
                    out_sb = opool.tile([P, EHALF], fp32, name="out_sb",
                                        tag="out_sb")
                    nc.vector.tensor_scalar_mul(out_sb[:], pso[:], recip[:])
                    nc.sync.dma_start(o_d[gb * P:(gb + 1) * P, :], out_sb[:])

    nc.compile()
    return nc


def _get_nc():
    if "nc" not in _CACHE:
        _CACHE["nc"] = _build_nc()
    return _CACHE["nc"]


def _shard_inputs(inputs_for_keys, inputs_for_values, inputs_for_queries,
                  WK, WQ, WV):
    xk = np.ascontiguousarray(np.asarray(inputs_for_keys, dtype=np.float32))
    xv = np.ascontiguousarray(np.asarray(inputs_for_values, dtype=np.float32))
    xq = np.ascontiguousarray(np.asarray(inputs_for_queries, dtype=np.float32))
    wk = np.ascontiguousarray(np.asarray(WK, dtype=np.float32))
    wq = np.ascontiguousarray(np.asarray(WQ, dtype=np.float32))
    wv = np.ascontiguousarray(np.asarray(WV, dtype=np.float32))
    in_maps = []
    for c in range(NCORES):
        b, h = divmod(c, 2)
        in_maps.append({
            "xk": xk[b],
            "xv": xv[b],
            "xq": xq[b],
            "wk": wk,
            "wq": wq,
            "wv": np.ascontiguousarray(wv[:, h * EHALF:(h + 1) * EHALF]),
        })
    return in_maps


def _assemble(results):
    out = np.empty((B, S, D), dtype=np.float32)
    for c in range(NCORES):
        b, h = divmod(c, 2)
        out[b, :, h * EHALF:(h + 1) * EHALF] = results[c]["o"]
    return out


def _run(in_maps, **kwargs):
    from concourse.bass_utils import run_bass_kernel_spmd
    nc = _get_nc()
    return run_bass_kernel_spmd(nc, in_maps, list(range(NCORES)), **kwargs)


def kernel(inputs_for_keys, inputs_for_values, inputs_for_queries,
           WK, WQ, WV):
    in_maps = _shard_inputs(inputs_for_keys, inputs_for_values,
                            inputs_for_queries, WK, WQ, WV)
    res = _run(in_maps)
    return _assemble(res.results)
